# revision 13
# baseline (speedup 1.0000x reference)
"""Multi-head self-attention Trainium2 kernel (8 NeuronCores).

Problem: x[4, 2048, 1024], H=16 heads, D=64. Sharding: core c handles
batch b = c // 2 and head-group hg = c % 2 (8 heads = 512 features).

All matmul operands are bf16 (shipped pre-converted from host); PSUM
accumulation stays fp32. Per-core math (F = 512 local features,
T = 2048 tokens, C = 1024):

  QT = (Wq_s.T @ x_b.T) + bq_s          [F, T]   feature-major, bf16
  KT = same with Wk_s                    [F, T]
  V65 = [x_b @ Wv_s + bv_s | 1]          [T, 8*(64+1)]  token-major
  per head-pair f (heads 2f, 2f+1 in partition halves of tile f):
    sc[:, 0:512]   = KT[f][0:64].T-tile  @ QT[f][0:64]    (PE rows 0-63)
    sc[:, 512:1024]= KT[f][64:128].T-tile@ QT[f][64:128]  (PE rows 64-127,
                     adjacent in program order -> concurrent row-groups)
    ex = exp(sc / 8) bf16                (one ACT op per head-pair tile)
    pvX[0:64] += V65_hX.T @ ex-half ; pvX[64] = softmax denominator
    renorm: dninv = 1/pv[64] (DVE), broadcast over 64 partitions
            (GpSimd partition_broadcast), attnT = pv * bcast (DVE)
  o_part = attnT.T @ Wo_s                [T, C]  fp32 out

QK projections for head-pair f+1 are emitted after attention f so the
scheduler threads them into PE gaps while ACT (exp) is the bottleneck;
they accumulate in a separate 1-bank PSUM tag to avoid slot contention
with the attention score tiles. The output projection is emitted per
query-chunk inside the last head-pair for the same reason.
Host: out[b] = o_part[2b] + o_part[2b+1] + bo.
"""

import sys

import numpy as np

if "/opt/trn_rl_repo" not in sys.path:
    sys.path.insert(0, "/opt/trn_rl_repo")

import ml_dtypes

import concourse.bass as bass
import concourse.mybir as mybir
import concourse.tile as tile
from concourse import bacc

F32 = mybir.dt.float32
BF16 = mybir.dt.bfloat16
AF = mybir.ActivationFunctionType

# Full-problem constants
B, N, C, H, D = 4, 2048, 1024, 16, 64
NCORES = 8
NH = 8          # heads per core
F = NH * D      # 512 core-local features
SCALE = 1.0 / 8.0  # 1/sqrt(D)


def build_attention_kernel(tok=N, cin=C, nh=NH):
    """Build the per-core Bass program. Returns the finalized Bass object."""
    f = nh * D
    c_t = cin // 128       # contraction tiles for projections (8)
    f_t = f // 128         # feature tiles = head pairs (4)
    t_t = tok // 128       # token tiles (16)
    n_qc = tok // 512      # query chunks (4)
    ocw = 512
    n_oc = cin // ocw      # output-proj column chunks (2)

    nc = bacc.Bacc("TRN2", target_bir_lowering=False, debug=False,
                   num_devices=NCORES)

    xT = nc.dram_tensor("xT", [cin, tok], BF16, kind="ExternalInput").ap()
    wq = nc.dram_tensor("wq", [cin, f], BF16, kind="ExternalInput").ap()
    wk = nc.dram_tensor("wk", [cin, f], BF16, kind="ExternalInput").ap()
    wv = nc.dram_tensor("wv", [cin, f], BF16, kind="ExternalInput").ap()
    bq = nc.dram_tensor("bq", [f, 1], F32, kind="ExternalInput").ap()
    bk = nc.dram_tensor("bk", [f, 1], F32, kind="ExternalInput").ap()
    bv = nc.dram_tensor("bv", [1, f], F32, kind="ExternalInput").ap()
    wo = nc.dram_tensor("wo", [f, cin], BF16, kind="ExternalInput").ap()
    o_part = nc.dram_tensor("o_part", [tok, cin], F32,
                            kind="ExternalOutput").ap()

    with tile.TileContext(nc) as tc:
        from contextlib import ExitStack
        with ExitStack() as ctx:
            # ---- persistent pools ----
            p_qk = ctx.enter_context(tc.tile_pool(name="p_qk", bufs=1))
            p_v = ctx.enter_context(tc.tile_pool(name="p_v", bufs=1))
            p_at = ctx.enter_context(tc.tile_pool(name="p_at", bufs=1))
            p_x = ctx.enter_context(tc.tile_pool(name="p_x", bufs=1))
            p_w = ctx.enter_context(tc.tile_pool(name="p_w", bufs=1))
            p_wo = ctx.enter_context(tc.tile_pool(name="p_wo", bufs=1))
            p_sm = ctx.enter_context(tc.tile_pool(name="p_sm", bufs=1))
            p_ex = ctx.enter_context(tc.tile_pool(name="p_ex", bufs=4))
            p_dn = ctx.enter_context(tc.tile_pool(name="p_dn", bufs=4))
            p_rb = ctx.enter_context(tc.tile_pool(name="p_rb", bufs=4))
            p_os = ctx.enter_context(tc.tile_pool(name="p_os", bufs=4))
            ps_sc = ctx.enter_context(
                tc.tile_pool(name="ps_sc", bufs=2, space="PSUM"))
            ps_pv = ctx.enter_context(
                tc.tile_pool(name="ps_pv", bufs=4, space="PSUM"))
            ps_pj = ps_pv

            # ---- input DMAs (ordered so QK0 can start earliest) ----
            xts = [p_x.tile([128, tok], BF16, tag=f"x{i}", name=f"xt{i}")
                   for i in range(c_t)]
            wq_s = [p_w.tile([128, f], BF16, tag=f"wq{i}", name=f"wq_s{i}")
                    for i in range(c_t)]
            wk_s = [p_w.tile([128, f], BF16, tag=f"wk{i}", name=f"wk_s{i}")
                    for i in range(c_t)]
            wv_s = [p_w.tile([128, f], BF16, tag=f"wv{i}", name=f"wv_s{i}")
                    for i in range(c_t)]
            bqs = p_sm.tile([128, f_t], F32, tag="bqs", name="bqs")
            bks = p_sm.tile([128, f_t], F32, tag="bks", name="bks")
            bvs = p_sm.tile([1, f], F32, tag="bvs", name="bvs")
            dma = nc.sync.dma_start
            dma(bqs[:, :], bq.rearrange("(a p) o -> p (a o)", p=128))
            dma(bks[:, :], bk.rearrange("(a p) o -> p (a o)", p=128))
            dma(bvs[:, :], bv[:, :])
            for i in range(c_t):
                dma(xts[i][:, :], xT[i * 128:(i + 1) * 128, :])
                dma(wq_s[i][:, :], wq[i * 128:(i + 1) * 128, :])
                dma(wk_s[i][:, :], wk[i * 128:(i + 1) * 128, :])
            for i in range(c_t):
                dma(wv_s[i][:, :], wv[i * 128:(i + 1) * 128, :])
            wo_s = [p_wo.tile([128, cin], BF16, tag=f"wo{i}", name=f"wo_s{i}")
                    for i in range(f_t)]
            for i in range(f_t):
                dma(wo_s[i][:, :], wo[i * 128:(i + 1) * 128, :])

            onesf = p_sm.tile([128, nh], BF16, tag="onesf", name="onesf")
            nc.vector.memset(onesf[:, :], 1.0)
            # warm the ACT exp table while DMAs run (saves the ~2.7us
            # table load at the first real exp)
            actwarm = p_sm.tile([1, 1], F32, tag="actwarm", name="actwarm")
            nc.scalar.activation(actwarm[:, :], onesf[0:1, 0:1], AF.Exp,
                                 scale=SCALE)
            # bv broadcast over all token partitions (used in V65 cast)
            bvs_bc = p_sm.tile([128, f], F32, tag="bvsbc", name="bvs_bc")
            nc.gpsimd.partition_broadcast(bvs_bc[:, :], bvs[:, :])

            QT = [p_qk.tile([128, tok], BF16, tag=f"qt{i}", name=f"QT{i}")
                  for i in range(f_t)]
            KT = [p_qk.tile([128, tok], BF16, tag=f"kt{i}", name=f"KT{i}")
                  for i in range(f_t)]
            V65 = [p_v.tile([128, nh * 65], BF16, tag=f"v{i}", name=f"V65_{i}")
                   for i in range(t_t)]
            attnT = [p_at.tile([128, tok], BF16, tag=f"at{i}", name=f"attnT{i}")
                     for i in range(f_t)]

            def emit_qk(ft, pool=None, ptag="pv"):
                """QT[ft], KT[ft] (feature-major) over all token chunks."""
                pool = pool or ps_pj
                for tch in range(tok // 512):
                    ts = slice(tch * 512, (tch + 1) * 512)
                    for (w_s, dst, bias) in ((wq_s, QT, bqs), (wk_s, KT, bks)):
                        ps = pool.tile([128, 512], F32, tag=ptag,
                                       name=f"psqk{ft}_{tch}_{dst[ft].name}")
                        for i in range(c_t):
                            nc.tensor.matmul(
                                ps[:, :],
                                w_s[i][:, ft * 128:(ft + 1) * 128],
                                xts[i][:, ts],
                                start=(i == 0), stop=(i == c_t - 1))
                        nc.vector.tensor_scalar_add(
                            dst[ft][:, ts], ps[:, :], bias[:, ft:ft + 1])

            def emit_v(gt, pool=None, ptag="pv"):
                """V65[gt]: token-major V + bias + ones column, bf16."""
                pool = pool or ps_pv
                tsl = slice(gt * 128, (gt + 1) * 128)
                psv = pool.tile([128, f], F32, tag="pv", name=f"psv{gt}")
                for i in range(c_t):
                    nc.tensor.matmul(
                        psv[:, :], xts[i][:, tsl], wv_s[i][:, :],
                        start=(i == 0), stop=(i == c_t - 1))
                v_dst = V65[gt].rearrange("p (h e) -> p h e", e=65)
                nc.vector.tensor_copy(v_dst[:, :, 64:65], onesf[:, 0:nh])
                nc.vector.tensor_add(
                    v_dst[:, :, 0:64],
                    psv.rearrange("p (h e) -> p h e", e=64)[:, :, :],
                    bvs_bc.rearrange("p (h e) -> p h e", e=64)[:, :, :])

            def emit_out_proj(tt, pool=None, ptag="pv"):
                """Output projection for token tile tt (128 tokens)."""
                pool = pool or ps_pj
                tsl = slice(tt * 128, (tt + 1) * 128)
                for oc in range(n_oc):
                    osl = slice(oc * ocw, (oc + 1) * ocw)
                    po = pool.tile([128, ocw], F32, tag=ptag,
                                   name=f"po{tt}_{oc}")
                    for i in range(f_t):
                        nc.tensor.matmul(po[:, :], attnT[i][:, tsl],
                                         wo_s[i][:, osl],
                                         start=(i == 0), stop=(i == f_t - 1))
                    ob = p_os.tile([128, ocw], F32, tag="os",
                                   name=f"ob{tt}_{oc}")
                    nc.vector.tensor_copy(ob[:, :], po[:, :])
                    nc.sync.dma_start(o_part[tsl, osl], ob[:, :])

            def emit_attn(ft):
                """Attention for head pair ft (heads 2ft, 2ft+1)."""
                hA, hB = 2 * ft, 2 * ft + 1
                for qc in range(n_qc):
                    qs = slice(qc * 512, (qc + 1) * 512)
                    pvA = ps_pv.tile([65, 512], F32, tag="pv",
                                     name=f"pvA{ft}_{qc}")
                    pvB = ps_pv.tile([65, 512], F32, tag="pv",
                                     name=f"pvB{ft}_{qc}")
                    for jt in range(t_t):
                        if ft == 0 and qc == 0:
                            # race V65 production ahead of its consumption
                            if jt % 2 == 0:
                                emit_v(jt, ps_pj, "pv")
                            else:
                                emit_v(jt, ps_pv, "pv")
                        js = slice(jt * 128, (jt + 1) * 128)
                        sc = ps_sc.tile([128, 1024], F32, tag="sc",
                                        name=f"sc{ft}_{qc}_{jt}")
                        # paired score matmuls in disjoint PE row groups
                        nc.tensor.matmul(sc[:, 0:512],
                                         KT[ft][0:64, js], QT[ft][0:64, qs],
                                         start=True, stop=True)
                        nc.tensor.matmul(sc[:, 512:1024],
                                         KT[ft][64:128, js], QT[ft][64:128, qs],
                                         start=True, stop=True)
                        ex = p_ex.tile([128, 1024], BF16, tag="ex",
                                       name=f"ex{ft}_{qc}_{jt}")
                        nc.scalar.activation(ex[:, :], sc[:, :], AF.Exp,
                                             scale=SCALE)
                        nc.tensor.matmul(pvA[:, :],
                                         V65[jt][:, hA * 65:(hA + 1) * 65],
                                         ex[:, 0:512],
                                         start=(jt == 0), stop=(jt == t_t - 1))
                        nc.tensor.matmul(pvB[:, :],
                                         V65[jt][:, hB * 65:(hB + 1) * 65],
                                         ex[:, 512:1024],
                                         start=(jt == 0), stop=(jt == t_t - 1))
                    for (pv, r0) in ((pvA, 0), (pvB, 64)):
                        dn = p_dn.tile([1, 512], F32, tag="dn",
                                       name=f"dn{ft}_{qc}_{r0}")
                        nc.vector.tensor_copy(dn[:, :], pv[64:65, :])
                        dninv = p_dn.tile([1, 512], F32, tag="dninv",
                                          name=f"dninv{ft}_{qc}_{r0}")
                        nc.vector.reciprocal_approx_fast(
                            out=dninv[:, :], in_=dn[:, :])
                        rpb = p_rb.tile([64, 512], F32, tag="rpb",
                                        name=f"rpb{ft}_{qc}_{r0}")
                        nc.gpsimd.partition_broadcast(rpb[:, :], dninv[:, :])
                        nc.vector.tensor_mul(attnT[ft][r0:r0 + 64, qs],
                                             pv[0:64, :], rpb[:, :])
                    if ft == f_t - 1:
                        # all heads done for these 512 tokens: project out
                        for tt in range(qc * 4, (qc + 1) * 4):
                            if qc == n_qc - 1:
                                emit_out_proj(tt, ps_sc, "sc")
                            else:
                                emit_out_proj(tt)

            # ---- emission order: QK0 first (on the idle sc slots) so
            # attention starts early; V65 raced inside attn f0/qc0; QK[f+1]
            # threads into attention-f PE gaps on the pj slot.
            emit_qk(0, ps_sc, "sc")
            for ft in range(f_t):
                emit_attn(ft)
                if ft + 1 < f_t:
                    emit_qk(ft + 1)

    nc.finalize()
    return nc


_NC_CACHE = {}


def _get_nc(key=(N, C, NH)):
    if key not in _NC_CACHE:
        _NC_CACHE[key] = build_attention_kernel(*key)
    return _NC_CACHE[key]


def make_in_maps(x, Wq, bq, Wk, bk, Wv, bv, Wo):
    """Shard full inputs into 8 per-core input maps (bf16 operands)."""
    bf = ml_dtypes.bfloat16
    xTs = [np.ascontiguousarray(x[b].T.astype(bf)) for b in range(B)]
    in_maps = []
    for c in range(NCORES):
        b, hg = divmod(c, 2)
        fs = slice(hg * F, (hg + 1) * F)
        in_maps.append({
            "xT": xTs[b],
            "wq": np.ascontiguousarray(Wq[:, fs].astype(bf)),
            "wk": np.ascontiguousarray(Wk[:, fs].astype(bf)),
            "wv": np.ascontiguousarray(Wv[:, fs].astype(bf)),
            "bq": np.ascontiguousarray(bq[fs].reshape(F, 1)),
            "bk": np.ascontiguousarray(bk[fs].reshape(F, 1)),
            "bv": np.ascontiguousarray(bv[fs].reshape(1, F)),
            "wo": np.ascontiguousarray(Wo[fs, :].astype(bf)),
        })
    return in_maps


def kernel(x, Wq, bq, Wk, bk, Wv, bv, Wo, bo, **_unused):
    from concourse.bass_utils import run_bass_kernel_spmd

    arrs = [np.asarray(a, dtype=np.float32)
            for a in (x, Wq, bq, Wk, bk, Wv, bv, Wo, bo)]
    x, Wq, bq, Wk, bk, Wv, bv, Wo, bo = arrs

    nc = _get_nc()
    in_maps = make_in_maps(x, Wq, bq, Wk, bk, Wv, bv, Wo)
    res = run_bass_kernel_spmd(nc, in_maps, core_ids=list(range(NCORES)))

    out = np.empty((B, N, C), dtype=np.float32)
    for b in range(B):
        out[b] = res.results[2 * b]["o_part"] + res.results[2 * b + 1]["o_part"] + bo
    return out


# revision 15
# speedup vs baseline: 1.0738x; 1.0738x over previous
"""Multi-head self-attention Trainium2 kernel (8 NeuronCores).

Problem: x[4, 2048, 1024], H=16 heads, D=64. Sharding: core c handles
batch b = c // 2 and head-group hg = c % 2 (8 heads = 512 features).

All matmul operands are bf16 (shipped pre-converted from host); PSUM
accumulation stays fp32. Per-core math (F = 512 local features,
T = 2048 tokens, C = 1024):

  QT = (Wq_s.T @ x_b.T) + bq_s          [F, T]   feature-major, bf16
  KT = same with Wk_s                    [F, T]
  V65 = [x_b @ Wv_s + bv_s | 1]          [T, 8*(64+1)]  token-major
  per head-pair f (heads 2f, 2f+1 in partition halves of tile f):
    sc[:, 0:512]   = KT[f][0:64].T-tile  @ QT[f][0:64]    (PE rows 0-63)
    sc[:, 512:1024]= KT[f][64:128].T-tile@ QT[f][64:128]  (PE rows 64-127,
                     adjacent in program order -> concurrent row-groups)
    ex = exp(sc / 8) bf16                (one ACT op per head-pair tile)
    pvX[0:64] += V65_hX.T @ ex-half ; pvX[64] = softmax denominator
    renorm: dninv = 1/pv[64] (DVE), broadcast over 64 partitions
            (GpSimd partition_broadcast), attnT = pv * bcast (DVE)
  o_part = attnT.T @ Wo_s                [T, C]  fp32 out

QK projections for head-pair f+1 are emitted after attention f so the
scheduler threads them into PE gaps while ACT (exp) is the bottleneck;
they accumulate in a separate 1-bank PSUM tag to avoid slot contention
with the attention score tiles. The output projection is emitted per
query-chunk inside the last head-pair for the same reason.
Host: out[b] = o_part[2b] + o_part[2b+1] + bo.
"""

import sys

import numpy as np

if "/opt/trn_rl_repo" not in sys.path:
    sys.path.insert(0, "/opt/trn_rl_repo")

import ml_dtypes

import concourse.bass as bass
import concourse.mybir as mybir
import concourse.tile as tile
from concourse import bacc

F32 = mybir.dt.float32
BF16 = mybir.dt.bfloat16
AF = mybir.ActivationFunctionType

# Full-problem constants
B, N, C, H, D = 4, 2048, 1024, 16, 64
NCORES = 8
NH = 8          # heads per core
F = NH * D      # 512 core-local features
SCALE = 1.0 / 8.0  # 1/sqrt(D)


def build_attention_kernel(tok=N, cin=C, nh=NH):
    """Build the per-core Bass program. Returns the finalized Bass object."""
    f = nh * D
    c_t = cin // 128       # contraction tiles for projections (8)
    f_t = f // 128         # feature tiles = head pairs (4)
    t_t = tok // 128       # token tiles (16)
    n_qc = tok // 512      # query chunks (4)
    ocw = 512
    n_oc = cin // ocw      # output-proj column chunks (2)

    nc = bacc.Bacc("TRN2", target_bir_lowering=False, debug=False,
                   num_devices=NCORES)

    xT = nc.dram_tensor("xT", [cin, tok], BF16, kind="ExternalInput").ap()
    wq = nc.dram_tensor("wq", [cin, f], BF16, kind="ExternalInput").ap()
    wk = nc.dram_tensor("wk", [cin, f], BF16, kind="ExternalInput").ap()
    wv = nc.dram_tensor("wv", [cin, f], BF16, kind="ExternalInput").ap()
    bq = nc.dram_tensor("bq", [f, 1], F32, kind="ExternalInput").ap()
    bk = nc.dram_tensor("bk", [f, 1], F32, kind="ExternalInput").ap()
    bv = nc.dram_tensor("bv", [1, f], F32, kind="ExternalInput").ap()
    wo = nc.dram_tensor("wo", [f, cin], BF16, kind="ExternalInput").ap()
    o_part = nc.dram_tensor("o_part", [tok, cin], F32,
                            kind="ExternalOutput").ap()

    with tile.TileContext(nc) as tc:
        from contextlib import ExitStack
        with ExitStack() as ctx:
            # ---- persistent pools ----
            p_qk = ctx.enter_context(tc.tile_pool(name="p_qk", bufs=1))
            p_v = ctx.enter_context(tc.tile_pool(name="p_v", bufs=1))
            p_at = ctx.enter_context(tc.tile_pool(name="p_at", bufs=1))
            p_x = ctx.enter_context(tc.tile_pool(name="p_x", bufs=1))
            p_w = ctx.enter_context(tc.tile_pool(name="p_w", bufs=1))
            p_wo = ctx.enter_context(tc.tile_pool(name="p_wo", bufs=1))
            p_sm = ctx.enter_context(tc.tile_pool(name="p_sm", bufs=1))
            p_ex = ctx.enter_context(tc.tile_pool(name="p_ex", bufs=6))
            p_dn = ctx.enter_context(tc.tile_pool(name="p_dn", bufs=4))
            p_rb = ctx.enter_context(tc.tile_pool(name="p_rb", bufs=4))
            p_os = ctx.enter_context(tc.tile_pool(name="p_os", bufs=4))
            ps_sc = ctx.enter_context(
                tc.tile_pool(name="ps_sc", bufs=2, space="PSUM"))
            ps_pv = ctx.enter_context(
                tc.tile_pool(name="ps_pv", bufs=3, space="PSUM"))
            ps_pj = ctx.enter_context(
                tc.tile_pool(name="ps_pj", bufs=1, space="PSUM"))

            # ---- input DMAs (ordered so QK0 can start earliest) ----
            xts = [p_x.tile([128, tok], BF16, tag=f"x{i}", name=f"xt{i}")
                   for i in range(c_t)]
            wq_s = [p_w.tile([128, f], BF16, tag=f"wq{i}", name=f"wq_s{i}")
                    for i in range(c_t)]
            wk_s = [p_w.tile([128, f], BF16, tag=f"wk{i}", name=f"wk_s{i}")
                    for i in range(c_t)]
            wv_s = [p_w.tile([128, f], BF16, tag=f"wv{i}", name=f"wv_s{i}")
                    for i in range(c_t)]
            bqs = p_sm.tile([128, f_t], F32, tag="bqs", name="bqs")
            bks = p_sm.tile([128, f_t], F32, tag="bks", name="bks")
            bvs = p_sm.tile([1, f], F32, tag="bvs", name="bvs")
            dma = nc.sync.dma_start
            dma(bqs[:, :], bq.rearrange("(a p) o -> p (a o)", p=128))
            dma(bks[:, :], bk.rearrange("(a p) o -> p (a o)", p=128))
            dma(bvs[:, :], bv[:, :])
            for i in range(c_t):
                dma(xts[i][:, :], xT[i * 128:(i + 1) * 128, :])
                dma(wq_s[i][:, :], wq[i * 128:(i + 1) * 128, :])
                dma(wk_s[i][:, :], wk[i * 128:(i + 1) * 128, :])
            for i in range(c_t):
                dma(wv_s[i][:, :], wv[i * 128:(i + 1) * 128, :])
            wo_s = [p_wo.tile([128, cin], BF16, tag=f"wo{i}", name=f"wo_s{i}")
                    for i in range(f_t)]
            for i in range(f_t):
                dma(wo_s[i][:, :], wo[i * 128:(i + 1) * 128, :])

            onesf = p_sm.tile([128, nh], BF16, tag="onesf", name="onesf")
            nc.vector.memset(onesf[:, :], 1.0)
            ones64 = p_sm.tile([1, 64], BF16, tag="ones64", name="ones64")
            nc.vector.memset(ones64[:, :], 1.0)
            # warm the ACT exp table while DMAs run (saves the ~2.7us
            # table load at the first real exp)
            actwarm = p_sm.tile([1, 1], F32, tag="actwarm", name="actwarm")
            nc.scalar.activation(actwarm[:, :], onesf[0:1, 0:1], AF.Exp,
                                 scale=SCALE)
            # bv broadcast over all token partitions (used in V65 cast)
            bvs_bc = p_sm.tile([128, f], F32, tag="bvsbc", name="bvs_bc")
            nc.gpsimd.partition_broadcast(bvs_bc[:, :], bvs[:, :])

            QT = [p_qk.tile([128, tok], BF16, tag=f"qt{i}", name=f"QT{i}")
                  for i in range(f_t)]
            KT = [p_qk.tile([128, tok], BF16, tag=f"kt{i}", name=f"KT{i}")
                  for i in range(f_t)]
            V65 = [p_v.tile([128, nh * 65], BF16, tag=f"v{i}", name=f"V65_{i}")
                   for i in range(t_t)]
            attnT = [p_at.tile([128, tok], BF16, tag=f"at{i}", name=f"attnT{i}")
                     for i in range(f_t)]

            def emit_qk(ft, pool=None, ptag="pj"):
                """QT[ft], KT[ft] (feature-major) over all token chunks."""
                pool = pool or ps_pj
                for tch in range(tok // 512):
                    ts = slice(tch * 512, (tch + 1) * 512)
                    for (w_s, dst, bias) in ((wq_s, QT, bqs), (wk_s, KT, bks)):
                        ps = pool.tile([128, 512], F32, tag=ptag,
                                       name=f"psqk{ft}_{tch}_{dst[ft].name}")
                        for i in range(c_t):
                            nc.tensor.matmul(
                                ps[:, :],
                                w_s[i][:, ft * 128:(ft + 1) * 128],
                                xts[i][:, ts],
                                start=(i == 0), stop=(i == c_t - 1))
                        nc.vector.tensor_scalar_add(
                            dst[ft][:, ts], ps[:, :], bias[:, ft:ft + 1])

            def emit_v(gt, pool=None, ptag="pv"):
                """V65[gt]: token-major V + bias + ones column, bf16."""
                pool = pool or ps_pv
                tsl = slice(gt * 128, (gt + 1) * 128)
                psv = pool.tile([128, f], F32, tag=ptag, name=f"psv{gt}")
                for i in range(c_t):
                    nc.tensor.matmul(
                        psv[:, :], xts[i][:, tsl], wv_s[i][:, :],
                        start=(i == 0), stop=(i == c_t - 1))
                v_dst = V65[gt].rearrange("p (h e) -> p h e", e=65)
                nc.vector.tensor_copy(v_dst[:, :, 64:65], onesf[:, 0:nh])
                nc.vector.tensor_add(
                    v_dst[:, :, 0:64],
                    psv.rearrange("p (h e) -> p h e", e=64)[:, :, :],
                    bvs_bc.rearrange("p (h e) -> p h e", e=64)[:, :, :])

            def emit_out_proj(tt, pool=None, ptag="pj"):
                """Output projection for token tile tt (128 tokens)."""
                pool = pool or ps_pj
                tsl = slice(tt * 128, (tt + 1) * 128)
                for oc in range(n_oc):
                    osl = slice(oc * ocw, (oc + 1) * ocw)
                    po = pool.tile([128, ocw], F32, tag=ptag,
                                   name=f"po{tt}_{oc}")
                    for i in range(f_t):
                        nc.tensor.matmul(po[:, :], attnT[i][:, tsl],
                                         wo_s[i][:, osl],
                                         start=(i == 0), stop=(i == f_t - 1))
                    ob = p_os.tile([128, ocw], F32, tag="os",
                                   name=f"ob{tt}_{oc}")
                    nc.vector.tensor_copy(ob[:, :], po[:, :])
                    nc.sync.dma_start(o_part[tsl, osl], ob[:, :])

            def emit_attn(ft):
                """Attention for head pair ft (heads 2ft, 2ft+1)."""
                hA, hB = 2 * ft, 2 * ft + 1
                for qc in range(n_qc):
                    qs = slice(qc * 512, (qc + 1) * 512)
                    pvA = ps_pv.tile([65, 512], F32, tag="pv",
                                     name=f"pvA{ft}_{qc}")
                    pvB = ps_pv.tile([65, 512], F32, tag="pv",
                                     name=f"pvB{ft}_{qc}")
                    for jt in range(t_t):
                        if ft == 0 and qc == 0:
                            # race V65 production ahead of its consumption
                            if jt % 2 == 0:
                                emit_v(jt, ps_pj, "pj")
                            else:
                                emit_v(jt, ps_pv, "pv")
                        js = slice(jt * 128, (jt + 1) * 128)
                        sc = ps_sc.tile([128, 1024], F32, tag="sc",
                                        name=f"sc{ft}_{qc}_{jt}")
                        # paired score matmuls in disjoint PE row groups
                        nc.tensor.matmul(sc[:, 0:512],
                                         KT[ft][0:64, js], QT[ft][0:64, qs],
                                         start=True, stop=True)
                        nc.tensor.matmul(sc[:, 512:1024],
                                         KT[ft][64:128, js], QT[ft][64:128, qs],
                                         start=True, stop=True)
                        ex = p_ex.tile([128, 1024], BF16, tag="ex",
                                       name=f"ex{ft}_{qc}_{jt}")
                        nc.scalar.activation(ex[:, :], sc[:, :], AF.Exp,
                                             scale=SCALE)
                        nc.tensor.matmul(pvA[:, :],
                                         V65[jt][:, hA * 65:(hA + 1) * 65],
                                         ex[:, 0:512],
                                         start=(jt == 0), stop=(jt == t_t - 1))
                        nc.tensor.matmul(pvB[:, :],
                                         V65[jt][:, hB * 65:(hB + 1) * 65],
                                         ex[:, 512:1024],
                                         start=(jt == 0), stop=(jt == t_t - 1))
                    for (pv, r0) in ((pvA, 0), (pvB, 64)):
                        dn = p_dn.tile([1, 512], F32, tag="dn",
                                       name=f"dn{ft}_{qc}_{r0}")
                        nc.vector.tensor_copy(dn[:, :], pv[64:65, :])
                        dninv = p_dn.tile([1, 512], F32, tag="dninv",
                                          name=f"dninv{ft}_{qc}_{r0}")
                        nc.vector.reciprocal_approx_fast(
                            out=dninv[:, :], in_=dn[:, :])
                        rpb = p_rb.tile([64, 512], F32, tag="rpb",
                                        name=f"rpb{ft}_{qc}_{r0}")
                        nc.gpsimd.partition_broadcast(rpb[:, :], dninv[:, :])
                        nc.vector.tensor_mul(attnT[ft][r0:r0 + 64, qs],
                                             pv[0:64, :], rpb[:, :])
                    if ft == f_t - 1:
                        # all heads done for these 512 tokens: project out
                        for tt in range(qc * 4, (qc + 1) * 4):
                            if qc == n_qc - 1:
                                emit_out_proj(tt, ps_sc, "sc")
                            else:
                                emit_out_proj(tt)

            # ---- emission order: QK0 first (on the idle sc slots) so
            # attention starts early; V65 raced inside attn f0/qc0; QK[f+1]
            # threads into attention-f PE gaps on the pj slot.
            emit_qk(0, ps_sc, "sc")
            for ft in range(f_t):
                emit_attn(ft)
                if ft + 1 < f_t:
                    emit_qk(ft + 1)

    nc.finalize()
    return nc


_NC_CACHE = {}


def _get_nc(key=(N, C, NH)):
    if key not in _NC_CACHE:
        _NC_CACHE[key] = build_attention_kernel(*key)
    return _NC_CACHE[key]


def make_in_maps(x, Wq, bq, Wk, bk, Wv, bv, Wo):
    """Shard full inputs into 8 per-core input maps (bf16 operands)."""
    bf = ml_dtypes.bfloat16
    xTs = [np.ascontiguousarray(x[b].T.astype(bf)) for b in range(B)]
    in_maps = []
    for c in range(NCORES):
        b, hg = divmod(c, 2)
        fs = slice(hg * F, (hg + 1) * F)
        in_maps.append({
            "xT": xTs[b],
            "wq": np.ascontiguousarray(Wq[:, fs].astype(bf)),
            "wk": np.ascontiguousarray(Wk[:, fs].astype(bf)),
            "wv": np.ascontiguousarray(Wv[:, fs].astype(bf)),
            "bq": np.ascontiguousarray(bq[fs].reshape(F, 1)),
            "bk": np.ascontiguousarray(bk[fs].reshape(F, 1)),
            "bv": np.ascontiguousarray(bv[fs].reshape(1, F)),
            "wo": np.ascontiguousarray(Wo[fs, :].astype(bf)),
        })
    return in_maps


def kernel(x, Wq, bq, Wk, bk, Wv, bv, Wo, bo, **_unused):
    from concourse.bass_utils import run_bass_kernel_spmd

    arrs = [np.asarray(a, dtype=np.float32)
            for a in (x, Wq, bq, Wk, bk, Wv, bv, Wo, bo)]
    x, Wq, bq, Wk, bk, Wv, bv, Wo, bo = arrs

    nc = _get_nc()
    in_maps = make_in_maps(x, Wq, bq, Wk, bk, Wv, bv, Wo)
    res = run_bass_kernel_spmd(nc, in_maps, core_ids=list(range(NCORES)))

    out = np.empty((B, N, C), dtype=np.float32)
    for b in range(B):
        out[b] = res.results[2 * b]["o_part"] + res.results[2 * b + 1]["o_part"] + bo
    return out
